# revision 4
# baseline (speedup 1.0000x reference)
"""Multi-head self-attention (RoPE) Trainium2 Bass kernel.

Shards batch (B=8) across 8 NeuronCores, one batch element per core.
Per core: fused qkv projection (fp16 matmuls), RoPE, flash-style attention
(scores row-tiled per head pair, exp on ACT with fused scale+mask-bias,
AV row-tiled by k-parity with a fused ones-column producing softmax
denominators), reciprocal-normalize, output projection.

KREPS body repetitions are emitted as a single hardware loop (tc.For_i)
so program size — and hence BIR->NEFF compile time — is independent of
the rep count; rep-count differencing then isolates true HW exec time.
"""
import os
import sys

# The kernel needs the 8 axon-tunneled NeuronCores visible to jax; a
# JAX_PLATFORMS=cpu pin (used by some harnesses for the reference) would
# hide them. Clear it before jax initializes through the concourse imports.
os.environ.pop("JAX_PLATFORMS", None)

sys.path.insert(0, "/opt/trn_rl_repo")

_REPS = int(os.environ.get("KREPS", "1"))
_PH = int(os.environ.get("KPHASES", "3"))

import numpy as np
from contextlib import ExitStack

import concourse.bass as bass
import concourse.tile as tile
from concourse import bacc, mybir
from concourse.tile import add_dep_helper

f32 = mybir.dt.float32
f16 = mybir.dt.float16
AF = mybir.ActivationFunctionType
ALU = mybir.AluOpType

B, L, DIM = 8, 1024, 512
NH, HD = 8, 64
SCALE = HD ** -0.5
NCORES = 8


def _build_nc(reps=None):
    if reps is None:
        reps = _REPS
    nc = bacc.Bacc("TRN2", target_bir_lowering=False, debug=False, enable_asserts=False)

    xT = nc.dram_tensor("xT", (DIM, L), f16, kind="ExternalInput")
    wq = nc.dram_tensor("wq", (DIM, 2 * DIM), f16, kind="ExternalInput")  # Q|K cols
    wv = nc.dram_tensor("wv", (DIM, DIM), f16, kind="ExternalInput")      # V cols
    wp = nc.dram_tensor("wp", (DIM, DIM), f16, kind="ExternalInput")
    cosT = nc.dram_tensor("cosT", (128, L), f16, kind="ExternalInput")
    sinT = nc.dram_tensor("sinT", (128, L), f16, kind="ExternalInput")
    bias = nc.dram_tensor("bias", (128, 8), f32, kind="ExternalInput")
    y = nc.dram_tensor("y", (L, DIM), f32, kind="ExternalOutput")

    with ExitStack() as ctx:
        tc = ctx.enter_context(tile.TileContext(nc))
        cst = ctx.enter_context(tc.tile_pool(name="cst", bufs=1))
        sc = ctx.enter_context(tc.tile_pool(name="sc", bufs=3))
        pTp = ctx.enter_context(tc.tile_pool(name="pTp", bufs=10))
        nrm = ctx.enter_context(tc.tile_pool(name="nrm", bufs=2))
        ysb = ctx.enter_context(tc.tile_pool(name="ysb", bufs=1))

        # ---- load inputs ----
        xT_all = cst.tile([128, 4 * L], f16, name="t", tag="xTall")
        wq_all = cst.tile([128, 4 * 2 * DIM], f16, name="t", tag="wqall")
        wv_all = cst.tile([128, 4 * DIM], f16, name="t", tag="wvall")
        wp_all = cst.tile([128, 4 * DIM], f16, name="t", tag="wpall")
        for big, dram, w in ((xT_all, xT, L), (wq_all, wq, 2 * DIM),
                             (wv_all, wv, DIM), (wp_all, wp, DIM)):
            nc.sync.dma_start(
                big[:].rearrange("p (kc w) -> p kc w", kc=4),
                dram[:].rearrange("(kc p) w -> p kc w", p=128))
        xT_sb = [xT_all[:, i * L:(i + 1) * L] for i in range(4)]
        wq_sb = [wq_all[:, i * 2 * DIM:(i + 1) * 2 * DIM] for i in range(4)]
        wv_sb = [wv_all[:, i * DIM:(i + 1) * DIM] for i in range(4)]
        wp_sb = [wp_all[:, i * DIM:(i + 1) * DIM] for i in range(4)]
        cos_sb = cst.tile([128, L], f16, name="t", tag="cos")
        sin_sb = cst.tile([128, L], f16, name="t", tag="sin")
        bias_sb = cst.tile([128, 8], f32, name="t", tag="bias")
        nc.sync.dma_start(cos_sb[:], cosT[:])
        nc.sync.dma_start(sin_sb[:], sinT[:])
        nc.sync.dma_start(bias_sb[:], bias[:])

        qkT = [cst.tile([64, L], f16, name="t", tag=f"qkT{m}") for m in range(16)]
        vaug = [cst.tile([128, NH * 128], f16, name="t", tag=f"vaug{i}") for i in range(8)]
        outT = [cst.tile([128, L], f16, name="t", tag=f"outT{c}") for c in range(4)]

        def emit_body(rep):
            untiled1 = []
            tiled = []
            # ---------- phase 1: qkv projection + RoPE ----------
            with tc.tile_pool(name=f"qkps{rep}", bufs=2, space="PSUM") as qk_ps, \
                 tc.tile_pool(name=f"vps{rep}", bufs=2, space="PSUM") as v_ps:
                for m in range(8):
                    ps = qk_ps.tile([128, L], f32, name="t", tag="qkps")
                    for kc in range(4):
                        for qb in range(2):
                            mm = nc.tensor.matmul(
                                ps[:, qb * 512:(qb + 1) * 512],
                                wq_sb[kc][:, m * 128:(m + 1) * 128],
                                xT_sb[kc][:, qb * 512:(qb + 1) * 512],
                                start=(kc == 0), stop=(kc == 3))
                            untiled1.append(mm)
                    qc = sc.tile([128, L], f16, name="t", tag="qc")
                    nc.vector.tensor_copy(qc[:], ps[:])
                    sw = sc.tile([128, L], f16, name="t", tag="sw")
                    for (do, so) in ((0, 32), (32, 0), (64, 96), (96, 64)):
                        nc.vector.tensor_copy(sw[do:do + 32, :], qc[so:so + 32, :])
                    q1 = sc.tile([128, L], f16, name="t", tag="q1")
                    nc.vector.tensor_mul(q1[:], qc[:], cos_sb[:])
                    q2 = sc.tile([128, L], f16, name="t", tag="q2")
                    nc.vector.tensor_mul(q2[:], sw[:], sin_sb[:])
                    nc.vector.tensor_add(qkT[2 * m][:], q1[0:64, :], q2[0:64, :])
                    nc.vector.tensor_add(qkT[2 * m + 1][:], q1[64:128, :], q2[64:128, :])

                for lb in range(8):
                    vps = v_ps.tile([128, DIM], f32, name="t", tag="vps")
                    for kc in range(4):
                        mm = nc.tensor.matmul(
                            vps[:],
                            xT_sb[kc][:, lb * 128:(lb + 1) * 128],
                            wv_sb[kc][:],
                            start=(kc == 0), stop=(kc == 3))
                        untiled1.append(mm)
                    ones_ap = vaug[lb][:].rearrange("p (h c) -> p h c", h=NH)[:, :, 64:128]
                    nc.vector.memset(ones_ap, 1.0)
                    out_ap = vaug[lb][:].rearrange("p (h c) -> p h c", h=NH)[:, :, 0:64]
                    in_ap = vps[:].rearrange("p (h c) -> p h c", h=NH)
                    nc.vector.tensor_copy(out_ap, in_ap)

            # ---------- phase 2: attention (row-tiled 64x128) ----------
            with tc.tile_pool(name=f"sps{rep}", bufs=1, space="PSUM") as s_ps, \
                 tc.tile_pool(name=f"avps{rep}", bufs=2, space="PSUM") as av_ps:

                def emit_scores(p):
                    pts = {}
                    for t in range(2):
                        h = 2 * p + t
                        QTc, KTc = qkT[h], qkT[8 + h]
                        for kb2 in range(4):
                            s = s_ps.tile([128, 2048], f32, name="t", tag="s")
                            for half in range(2):
                                kb = kb2 * 2 + half
                                for qb in range(2):
                                    mm = nc.tensor.matmul(
                                        s[:, half * 1024 + qb * 512:
                                           half * 1024 + (qb + 1) * 512],
                                        KTc[:, kb * 128:(kb + 1) * 128],
                                        QTc[:, qb * 512:(qb + 1) * 512],
                                        start=True, stop=True)
                                    tiled.append(mm)
                            pt = pTp.tile([128, 2048], f16, name="t", tag="pT")
                            nc.scalar.activation(pt[:], s[:], AF.Exp,
                                                 bias=bias_sb[:, 2 * kb2:2 * kb2 + 1],
                                                 scale=SCALE)
                            pts[(t, kb2)] = pt
                    return pts

                def emit_av_norm(p, pts):
                    for t in range(2):
                        h = 2 * p + t
                        X = av_ps.tile([128, L], f32, name="t", tag="avX")
                        for kc in range(8):
                            pt = pts[(t, kc // 2)]
                            off = (kc % 2) * 1024
                            va = vaug[kc][:, h * 128:(h + 1) * 128]
                            for qb in range(2):
                                q0, q1_ = off + qb * 512, off + (qb + 1) * 512
                                mm = nc.tensor.matmul(
                                    X[:, qb * 512:(qb + 1) * 512], va, pt[:, q0:q1_],
                                    start=(kc == 0), stop=(kc == 7))
                                tiled.append(mm)
                        D = nrm.tile([64, L], f32, name="t", tag="D")
                        nc.vector.tensor_copy(D[:], X[64:128, :])
                        R = nrm.tile([64, L], f32, name="t", tag="R")
                        nc.vector.reciprocal_approx_fast(R[:], D[:])
                        if t == 0:
                            nc.vector.tensor_mul(outT[p][0:64, :], X[0:64, :], R[:])
                        else:
                            tmp = nrm.tile([64, L], f16, name="t", tag="tmp")
                            nc.vector.tensor_mul(tmp[:], X[0:64, :], R[:])
                            nc.sync.dma_start(outT[p][64:128, :], tmp[:])

                prev = None
                for p in range(4 if _PH >= 2 else 0):
                    pts = emit_scores(p)
                    if prev is not None and _PH >= 3:
                        emit_av_norm(prev[0], prev[1])
                    prev = (p, pts)
                if prev is not None and _PH >= 3:
                    emit_av_norm(prev[0], prev[1])

            # ---------- phase 3: output projection ----------
            with tc.tile_pool(name=f"yps{rep}", bufs=2, space="PSUM") as y_ps:
                yall = ysb.tile([128, 8 * DIM], f32, name="t", tag="yall")
                for lb in range(8 if _PH >= 3 else 0):
                    yp = y_ps.tile([128, DIM], f32, name="t", tag="yps")
                    for c in range(4):
                        mm = nc.tensor.matmul(
                            yp[:],
                            outT[c][:, lb * 128:(lb + 1) * 128],
                            wp_sb[c][:],
                            start=(c == 0), stop=(c == 3))
                    nc.vector.tensor_copy(yall[:, lb * DIM:(lb + 1) * DIM], yp[:])
                nc.sync.dma_start(
                    y[:].rearrange("(lb p) d -> p lb d", p=128),
                    yall[:].rearrange("p (lb d) -> p lb d", lb=8))

        if reps == 1:
            emit_body(0)
        else:
            with tc.For_i(0, reps) as _i:
                emit_body(0)

    nc.compile()
    return nc


def _rope_tables():
    inv_freq = 1.0 / (10000.0 ** (np.arange(0, HD, 2, dtype=np.float32) / HD))
    t = np.arange(L, dtype=np.float32)
    freqs = np.outer(t, inv_freq)                      # (L, 32)
    emb = np.concatenate([freqs, freqs], axis=-1)      # (L, 64)
    cos = np.cos(emb).T                                # (64, L)
    sin = np.sin(emb).T                                # (64, L)
    sign = np.where(np.arange(HD) < HD // 2, -1.0, 1.0)[:, None].astype(np.float32)
    sin_s = sin * sign
    cosT = np.tile(cos, (2, 1)).astype(np.float16)     # (128, L)
    sinT = np.tile(sin_s, (2, 1)).astype(np.float16)   # (128, L)
    return cosT, sinT


def _make_runner(nc, n_cores=NCORES):
    """Reusable jitted SPMD runner (replicates run_bass_kernel_spmd's axon
    redirect, but caches the jax.jit so repeat calls skip retrace +
    BIR->NEFF recompile + executable reload)."""
    import jax
    from jax.sharding import Mesh, PartitionSpec
    from jax.experimental.shard_map import shard_map
    from concourse import bass2jax

    bass2jax.install_neuronx_cc_hook()
    partition_name = nc.partition_id_tensor.name if nc.partition_id_tensor else None
    in_names, out_names, out_avals, zero_outs = [], [], [], []
    for alloc in nc.m.functions[0].allocations:
        if not isinstance(alloc, mybir.MemoryLocationSet):
            continue
        name = alloc.memorylocations[0].name
        if alloc.kind == "ExternalInput":
            if name != partition_name:
                in_names.append(name)
        elif alloc.kind == "ExternalOutput":
            shape = tuple(alloc.tensor_shape)
            dtype = mybir.dt.np(alloc.dtype)
            out_names.append(name)
            out_avals.append(jax.core.ShapedArray(shape, dtype))
            zero_outs.append(np.zeros(shape, dtype))
    n_params = len(in_names)
    n_outs = len(out_avals)
    all_in = list(in_names) + list(out_names)
    if partition_name is not None:
        all_in.append(partition_name)

    def _body(*args):
        operands = list(args)
        if partition_name is not None:
            operands.append(bass2jax.partition_id_tensor())
        outs = bass2jax._bass_exec_p.bind(
            *operands,
            out_avals=tuple(out_avals),
            in_names=tuple(all_in),
            out_names=tuple(out_names),
            lowering_input_output_aliases=(),
            sim_require_finite=True,
            sim_require_nnan=True,
            nc=nc,
        )
        return tuple(outs)

    devices = jax.devices()[:n_cores]
    mesh = Mesh(np.asarray(devices), ("core",))
    in_specs = (PartitionSpec("core"),) * (n_params + n_outs)
    out_specs = (PartitionSpec("core"),) * n_outs
    fn = jax.jit(
        shard_map(_body, mesh=mesh, in_specs=in_specs, out_specs=out_specs,
                  check_rep=False),
        keep_unused=True,
    )
    concat_zeros = [np.zeros((n_cores * z.shape[0], *z.shape[1:]), z.dtype)
                    for z in zero_outs]

    def run(in_maps):
        per_core = [[np.asarray(m[name]) for name in in_names] for m in in_maps]
        concat_in = [np.concatenate([per_core[c][i] for c in range(n_cores)], axis=0)
                     for i in range(n_params)]
        out_arrs = fn(*concat_in, *concat_zeros)
        return [
            {name: np.asarray(out_arrs[i]).reshape(n_cores, *out_avals[i].shape)[c]
             for i, name in enumerate(out_names)}
            for c in range(n_cores)
        ]

    run.fn = fn
    run.in_names = in_names
    run.concat_zeros = concat_zeros
    return run


def _make_runner_dev(nc, in_maps, n_cores=NCORES):
    """Timing variant of _make_runner: inputs pre-transferred to device so
    steady-state calls measure dispatch + device execution only."""
    import jax
    from jax.sharding import Mesh, PartitionSpec, NamedSharding

    run = _make_runner(nc, n_cores)
    fn = run.fn
    in_names = run.in_names
    concat_zeros = run.concat_zeros
    per_core = [[np.asarray(m[name]) for name in in_names] for m in in_maps]
    concat_in = [np.concatenate([per_core[c][i] for c in range(n_cores)], axis=0)
                 for i in range(len(in_names))]
    devices = jax.devices()[:n_cores]
    mesh = Mesh(np.asarray(devices), ("core",))
    sh = NamedSharding(mesh, PartitionSpec("core"))
    dev_args = [jax.device_put(a, sh) for a in concat_in + concat_zeros]

    def run_dev():
        outs = fn(*dev_args)
        jax.block_until_ready(outs)
        return outs

    return run_dev


_NC = None
_RUN = None


def _get_runner():
    global _NC, _RUN
    if _RUN is None:
        _NC = _build_nc()
        try:
            _RUN = _make_runner(_NC)
        except Exception:
            from concourse.bass_utils import run_bass_kernel_spmd

            def _RUN_fallback(in_maps):
                res = run_bass_kernel_spmd(_NC, in_maps,
                                           core_ids=list(range(NCORES)))
                return res.results
            _RUN = _RUN_fallback
    return _RUN


def kernel(x, mask, w_qkv, w_proj):
    x = np.asarray(x, dtype=np.float32)
    mask = np.asarray(mask)
    w_qkv = np.asarray(w_qkv, dtype=np.float32)
    w_proj = np.asarray(w_proj, dtype=np.float32)

    run = _get_runner()
    cosT, sinT = _rope_tables()

    wq = np.ascontiguousarray(w_qkv[:, :2 * DIM]).astype(np.float16)
    wv = np.ascontiguousarray(w_qkv[:, 2 * DIM:]).astype(np.float16)
    wp = w_proj.astype(np.float16)

    in_maps = []
    for b in range(NCORES):
        xTb = np.ascontiguousarray(x[b].T).astype(np.float16)      # (512, 1024)
        bias_b = np.where(mask[b].reshape(8, 128).T, 0.0, -1e9).astype(np.float32)
        in_maps.append({
            "xT": xTb, "wq": wq, "wv": wv, "wp": wp,
            "cosT": cosT, "sinT": sinT, "bias": bias_b,
        })

    results = run(in_maps)
    out = np.stack([results[c]["y"] for c in range(NCORES)], axis=0)
    return out.astype(np.float32)


# revision 12
# speedup vs baseline: 1.1199x; 1.1199x over previous
"""Multi-head self-attention (RoPE) Trainium2 Bass kernel.

Shards batch (B=8) across 8 NeuronCores, one batch element per core.
Per core: fused qkv projection (fp16 matmuls), RoPE, flash-style attention
(scores row-tiled per head pair, exp on ACT with fused scale+mask-bias,
AV row-tiled by k-parity with a fused ones-column producing softmax
denominators), reciprocal-normalize, output projection.

KREPS body repetitions are emitted as a single hardware loop (tc.For_i)
so program size — and hence BIR->NEFF compile time — is independent of
the rep count; rep-count differencing then isolates true HW exec time.
"""
import os
import sys

# The kernel needs the 8 axon-tunneled NeuronCores visible to jax; a
# JAX_PLATFORMS=cpu pin (used by some harnesses for the reference) would
# hide them. Clear it before jax initializes through the concourse imports.
os.environ.pop("JAX_PLATFORMS", None)

sys.path.insert(0, "/opt/trn_rl_repo")

_REPS = int(os.environ.get("KREPS", "1"))
_DBG = os.environ.get("KDBG", "")

import numpy as np
from contextlib import ExitStack

import concourse.bass as bass
import concourse.tile as tile
from concourse import bacc, mybir
from concourse.tile import add_dep_helper

f32 = mybir.dt.float32
f16 = mybir.dt.float16
AF = mybir.ActivationFunctionType
ALU = mybir.AluOpType

B, L, DIM = 8, 1024, 512
NH, HD = 8, 64
SCALE = HD ** -0.5
NCORES = 8


def _build_nc(reps=None):
    if reps is None:
        reps = _REPS
    nc = bacc.Bacc("TRN2", target_bir_lowering=False, debug=False, enable_asserts=False)

    xT = nc.dram_tensor("xT", (DIM, L), f16, kind="ExternalInput")
    wq = nc.dram_tensor("wq", (DIM, 2 * DIM), f16, kind="ExternalInput")  # Q|K cols
    wv = nc.dram_tensor("wv", (DIM, DIM), f16, kind="ExternalInput")      # V cols
    wp = nc.dram_tensor("wp", (DIM, DIM), f16, kind="ExternalInput")
    cosT = nc.dram_tensor("cosT", (128, L), f16, kind="ExternalInput")
    sinT = nc.dram_tensor("sinT", (128, L), f16, kind="ExternalInput")
    bias = nc.dram_tensor("bias", (128, 8), f32, kind="ExternalInput")
    pmat = nc.dram_tensor("pmat", (128, 128), f16, kind="ExternalInput")
    y = nc.dram_tensor("y", (L, DIM), f32, kind="ExternalOutput")

    with ExitStack() as ctx:
        tc = ctx.enter_context(tile.TileContext(nc))
        cst = ctx.enter_context(tc.tile_pool(name="cst", bufs=1))
        sc = ctx.enter_context(tc.tile_pool(name="sc", bufs=3))
        pTp = ctx.enter_context(tc.tile_pool(name="pTp", bufs=24))
        nrm = ctx.enter_context(tc.tile_pool(name="nrm", bufs=2))
        ysb = ctx.enter_context(tc.tile_pool(name="ysb", bufs=1))

        # ---- load inputs ----
        xT_all = cst.tile([128, 4 * L], f16, name="t", tag="xTall")
        wq_all = cst.tile([128, 4 * 2 * DIM], f16, name="t", tag="wqall")
        wv_all = cst.tile([128, 4 * DIM], f16, name="t", tag="wvall")
        wp_all = cst.tile([128, 4 * DIM], f16, name="t", tag="wpall")
        for big, dram, w in ((xT_all, xT, L), (wq_all, wq, 2 * DIM),
                             (wv_all, wv, DIM), (wp_all, wp, DIM)):
            nc.sync.dma_start(
                big[:].rearrange("p (kc w) -> p kc w", kc=4),
                dram[:].rearrange("(kc p) w -> p kc w", p=128))
        xT_sb = [xT_all[:, i * L:(i + 1) * L] for i in range(4)]
        wq_sb = [wq_all[:, i * 2 * DIM:(i + 1) * 2 * DIM] for i in range(4)]
        wv_sb = [wv_all[:, i * DIM:(i + 1) * DIM] for i in range(4)]
        wp_sb = [wp_all[:, i * DIM:(i + 1) * DIM] for i in range(4)]
        cos_sb = cst.tile([128, L], f16, name="t", tag="cos")
        sin_sb = cst.tile([128, L], f16, name="t", tag="sin")
        bias_sb = cst.tile([128, 8], f32, name="t", tag="bias")
        pmat_sb = cst.tile([128, 128], f16, name="t", tag="pmat")
        nc.sync.dma_start(cos_sb[:], cosT[:])
        nc.sync.dma_start(sin_sb[:], sinT[:])
        nc.sync.dma_start(bias_sb[:], bias[:])
        nc.sync.dma_start(pmat_sb[:], pmat[:])

        # qkTfull[m]: QK-col block m after RoPE; partitions 0:64 = head 2m,
        # 64:128 = head 2m+1 (m 0-3 = Q heads 0-7, m 4-7 = K heads 0-7)
        qkTfull = [cst.tile([128, L], f16, name="t", tag=f"qkT{m}") for m in range(8)]
        vaug = [cst.tile([128, NH * 128], f16, name="t", tag=f"vaug{i}") for i in range(8)]
        outT = [cst.tile([128, L], f16, name="t", tag=f"outT{c}") for c in range(4)]

        # ones columns of vaug are rep-invariant: set them once, pre-loop
        for lb in range(8):
            ones_ap = vaug[lb][:].rearrange("p (h c) -> p h c", h=NH)[:, :, 64:128]
            nc.vector.memset(ones_ap, 1.0)

        def emit_body(rep):
            # ---------- phase 1: qkv projection + RoPE ----------
            # Per m-block: PE does 8 qkv matmuls + 2 rotate-half permutation
            # matmuls; ACT drains psum->f16; DVE does cos/sin muls + add.
            # P-matmul/DVE chain for block m is emitted under block m+1's
            # matmuls (software pipeline) so PE never waits on ACT.
            with tc.tile_pool(name=f"qkps{rep}", bufs=2, space="PSUM") as qk_ps, \
                 tc.tile_pool(name=f"swps{rep}", bufs=1, space="PSUM") as sw_ps, \
                 tc.tile_pool(name=f"vps{rep}", bufs=2, space="PSUM") as v_ps:

                def rope_tail(m, qc):
                    swp = sw_ps.tile([128, L], f32, name="t", tag="swps")
                    for qb in range(2):
                        nc.tensor.matmul(
                            swp[:, qb * 512:(qb + 1) * 512],
                            pmat_sb[:],
                            qc[:, qb * 512:(qb + 1) * 512],
                            start=True, stop=True)
                    q1 = sc.tile([128, L], f16, name="t", tag="q1")
                    nc.vector.tensor_mul(q1[:], qc[:], cos_sb[:])
                    q2 = sc.tile([128, L], f16, name="t", tag="q2")
                    if "q" in _DBG:
                        swsb = sc.tile([128, L], f16, name="t", tag="swsb")
                        nc.scalar.activation(swsb[:], swp[:], AF.Copy)
                        nc.vector.tensor_mul(q2[:], swsb[:], sin_sb[:])
                    else:
                        nc.vector.tensor_mul(q2[:], swp[:], sin_sb[:])
                    nc.vector.tensor_add(qkTfull[m][:], q1[:], q2[:])

                prev = None  # (m, qc)
                for m in (0, 4, 1, 5, 2, 6, 3, 7):
                    ps = qk_ps.tile([128, L], f32, name="t", tag="qkps")
                    for kc in range(4):
                        for qb in range(2):
                            nc.tensor.matmul(
                                ps[:, qb * 512:(qb + 1) * 512],
                                wq_sb[kc][:, m * 128:(m + 1) * 128],
                                xT_sb[kc][:, qb * 512:(qb + 1) * 512],
                                start=(kc == 0), stop=(kc == 3))
                    qc = sc.tile([128, L], f16, name="t", tag="qc")
                    nc.scalar.activation(qc[:], ps[:], AF.Copy)
                    if prev is not None:
                        rope_tail(*prev)
                    prev = (m, qc)

                for lb in range(8):
                    vps = v_ps.tile([128, DIM], f32, name="t", tag="vps")
                    for kc in range(4):
                        nc.tensor.matmul(
                            vps[:],
                            xT_sb[kc][:, lb * 128:(lb + 1) * 128],
                            wv_sb[kc][:],
                            start=(kc == 0), stop=(kc == 3))
                    if lb == 0:
                        rope_tail(*prev)
                        prev = None
                    out_ap = vaug[lb][:].rearrange("p (h c) -> p h c", h=NH)[:, :, 0:64]
                    in_ap = vps[:].rearrange("p (h c) -> p h c", h=NH)
                    if "v" in _DBG:
                        nc.vector.tensor_copy(out_ap, in_ap)
                    else:
                        nc.scalar.activation(out_ap, in_ap, AF.Copy)

            # ---------- phase 2: attention ----------
            # scores(p) tiles [128 k-pos, 1024 q] double-buffered; ACT does
            # exp with per-k-block mask bias; AV(p-1) matmul pairs are
            # interleaved between scores tiles to keep PE busy while ACT
            # drains; denominators come from vaug's fused ones columns.
            with tc.tile_pool(name=f"sps{rep}", bufs=2, space="PSUM") as s_ps, \
                 tc.tile_pool(name=f"avps{rep}", bufs=2, space="PSUM") as av_ps:

                def q_ap(h):
                    return qkTfull[h // 2][64 * (h % 2):64 * (h % 2) + 64, :]

                def k_ap(h):
                    return qkTfull[4 + h // 2][64 * (h % 2):64 * (h % 2) + 64, :]

                def av_steps(p, pts):
                    """Generator: 16 steps of 2 AV matmuls; normalize after
                    each head completes."""
                    for t in range(2):
                        h = 2 * p + t
                        X = av_ps.tile([128, L], f32, name="t", tag="avX")
                        for kc in range(8):
                            pt = pts[(t, kc)]
                            va = vaug[kc][:, h * 128:(h + 1) * 128]
                            for qb in range(2):
                                nc.tensor.matmul(
                                    X[:, qb * 512:(qb + 1) * 512], va,
                                    pt[:, qb * 512:(qb + 1) * 512],
                                    start=(kc == 0), stop=(kc == 7))
                            yield
                        D = nrm.tile([64, L], f32, name="t", tag="D")
                        nc.vector.tensor_copy(D[:], X[64:128, :])
                        R = nrm.tile([64, L], f32, name="t", tag="R")
                        nc.vector.reciprocal_approx_fast(R[:], D[:])
                        if t == 0:
                            nc.vector.tensor_mul(outT[p][0:64, :], X[0:64, :], R[:])
                        else:
                            tmp = nrm.tile([64, L], f16, name="t", tag="tmp")
                            nc.vector.tensor_mul(tmp[:], X[0:64, :], R[:])
                            nc.sync.dma_start(outT[p][64:128, :], tmp[:])

                prev = None  # generator of av steps for p-1
                for p in range(4):
                    pts = {}
                    for j in range(16):
                        t, kb = divmod(j, 8)
                        h = 2 * p + t
                        QT, KT = q_ap(h), k_ap(h)
                        s = s_ps.tile([128, 1024], f32, name="t", tag="s")
                        for qb in range(2):
                            nc.tensor.matmul(
                                s[:, qb * 512:(qb + 1) * 512],
                                KT[:, kb * 128:(kb + 1) * 128],
                                QT[:, qb * 512:(qb + 1) * 512],
                                start=True, stop=True)
                        pt = pTp.tile([128, 1024], f16, name="t", tag="pT")
                        nc.scalar.activation(pt[:], s[:], AF.Exp,
                                             bias=bias_sb[:, kb:kb + 1],
                                             scale=SCALE)
                        pts[(t, kb)] = pt
                        if prev is not None:
                            next(prev, None)
                    if prev is not None:
                        for _ in prev:   # finish av(p-1) tail (t=1 normalize)
                            pass
                    prev = av_steps(p, pts)
                for _ in prev:
                    pass

            # ---------- phase 3: output projection ----------
            with tc.tile_pool(name=f"yps{rep}", bufs=2, space="PSUM") as y_ps:
                yall = ysb.tile([128, 8 * DIM], f32, name="t", tag="yall")
                for lb in range(8):
                    yp = y_ps.tile([128, DIM], f32, name="t", tag="yps")
                    for c in range(4):
                        nc.tensor.matmul(
                            yp[:],
                            outT[c][:, lb * 128:(lb + 1) * 128],
                            wp_sb[c][:],
                            start=(c == 0), stop=(c == 3))
                    if "y" in _DBG:
                        nc.vector.tensor_copy(yall[:, lb * DIM:(lb + 1) * DIM], yp[:])
                    else:
                        nc.scalar.activation(yall[:, lb * DIM:(lb + 1) * DIM], yp[:],
                                             AF.Copy)
                nc.sync.dma_start(
                    y[:].rearrange("(lb p) d -> p lb d", p=128),
                    yall[:].rearrange("p (lb d) -> p lb d", lb=8))

        if reps == 1:
            emit_body(0)
        else:
            with tc.For_i(0, reps) as _i:
                emit_body(0)

    nc.compile()
    return nc


def _perm_matrix():
    """[128,128] f16: out[i] = in[(i%64+32)%64 + 64*(i//64)] — rotate_half's
    partition permutation (sign lives in sinT)."""
    P = np.zeros((128, 128), dtype=np.float16)
    for i in range(128):
        src = (i % 64 + 32) % 64 + 64 * (i // 64)
        P[src, i] = 1.0
    return P


def _rope_tables():
    inv_freq = 1.0 / (10000.0 ** (np.arange(0, HD, 2, dtype=np.float32) / HD))
    t = np.arange(L, dtype=np.float32)
    freqs = np.outer(t, inv_freq)                      # (L, 32)
    emb = np.concatenate([freqs, freqs], axis=-1)      # (L, 64)
    cos = np.cos(emb).T                                # (64, L)
    sin = np.sin(emb).T                                # (64, L)
    sign = np.where(np.arange(HD) < HD // 2, -1.0, 1.0)[:, None].astype(np.float32)
    sin_s = sin * sign
    cosT = np.tile(cos, (2, 1)).astype(np.float16)     # (128, L)
    sinT = np.tile(sin_s, (2, 1)).astype(np.float16)   # (128, L)
    return cosT, sinT


def _make_runner(nc, n_cores=NCORES):
    """Reusable jitted SPMD runner (replicates run_bass_kernel_spmd's axon
    redirect, but caches the jax.jit so repeat calls skip retrace +
    BIR->NEFF recompile + executable reload)."""
    import jax
    from jax.sharding import Mesh, PartitionSpec
    from jax.experimental.shard_map import shard_map
    from concourse import bass2jax

    bass2jax.install_neuronx_cc_hook()
    partition_name = nc.partition_id_tensor.name if nc.partition_id_tensor else None
    in_names, out_names, out_avals, zero_outs = [], [], [], []
    for alloc in nc.m.functions[0].allocations:
        if not isinstance(alloc, mybir.MemoryLocationSet):
            continue
        name = alloc.memorylocations[0].name
        if alloc.kind == "ExternalInput":
            if name != partition_name:
                in_names.append(name)
        elif alloc.kind == "ExternalOutput":
            shape = tuple(alloc.tensor_shape)
            dtype = mybir.dt.np(alloc.dtype)
            out_names.append(name)
            out_avals.append(jax.core.ShapedArray(shape, dtype))
            zero_outs.append(np.zeros(shape, dtype))
    n_params = len(in_names)
    n_outs = len(out_avals)
    all_in = list(in_names) + list(out_names)
    if partition_name is not None:
        all_in.append(partition_name)

    def _body(*args):
        operands = list(args)
        if partition_name is not None:
            operands.append(bass2jax.partition_id_tensor())
        outs = bass2jax._bass_exec_p.bind(
            *operands,
            out_avals=tuple(out_avals),
            in_names=tuple(all_in),
            out_names=tuple(out_names),
            lowering_input_output_aliases=(),
            sim_require_finite=True,
            sim_require_nnan=True,
            nc=nc,
        )
        return tuple(outs)

    devices = jax.devices()[:n_cores]
    mesh = Mesh(np.asarray(devices), ("core",))
    in_specs = (PartitionSpec("core"),) * (n_params + n_outs)
    out_specs = (PartitionSpec("core"),) * n_outs
    fn = jax.jit(
        shard_map(_body, mesh=mesh, in_specs=in_specs, out_specs=out_specs,
                  check_rep=False),
        keep_unused=True,
    )
    concat_zeros = [np.zeros((n_cores * z.shape[0], *z.shape[1:]), z.dtype)
                    for z in zero_outs]

    def run(in_maps):
        per_core = [[np.asarray(m[name]) for name in in_names] for m in in_maps]
        concat_in = [np.concatenate([per_core[c][i] for c in range(n_cores)], axis=0)
                     for i in range(n_params)]
        out_arrs = fn(*concat_in, *concat_zeros)
        return [
            {name: np.asarray(out_arrs[i]).reshape(n_cores, *out_avals[i].shape)[c]
             for i, name in enumerate(out_names)}
            for c in range(n_cores)
        ]

    run.fn = fn
    run.in_names = in_names
    run.concat_zeros = concat_zeros
    return run


def _make_runner_dev(nc, in_maps, n_cores=NCORES):
    """Timing variant of _make_runner: inputs pre-transferred to device so
    steady-state calls measure dispatch + device execution only."""
    import jax
    from jax.sharding import Mesh, PartitionSpec, NamedSharding

    run = _make_runner(nc, n_cores)
    fn = run.fn
    in_names = run.in_names
    concat_zeros = run.concat_zeros
    per_core = [[np.asarray(m[name]) for name in in_names] for m in in_maps]
    concat_in = [np.concatenate([per_core[c][i] for c in range(n_cores)], axis=0)
                 for i in range(len(in_names))]
    devices = jax.devices()[:n_cores]
    mesh = Mesh(np.asarray(devices), ("core",))
    sh = NamedSharding(mesh, PartitionSpec("core"))
    dev_args = [jax.device_put(a, sh) for a in concat_in + concat_zeros]

    def run_dev():
        outs = fn(*dev_args)
        jax.block_until_ready(outs)
        return outs

    return run_dev


_NC = None
_RUN = None


def _get_runner():
    global _NC, _RUN
    if _RUN is None:
        _NC = _build_nc()
        try:
            _RUN = _make_runner(_NC)
        except Exception:
            from concourse.bass_utils import run_bass_kernel_spmd

            def _RUN_fallback(in_maps):
                res = run_bass_kernel_spmd(_NC, in_maps,
                                           core_ids=list(range(NCORES)))
                return res.results
            _RUN = _RUN_fallback
    return _RUN


def kernel(x, mask, w_qkv, w_proj):
    x = np.asarray(x, dtype=np.float32)
    mask = np.asarray(mask)
    w_qkv = np.asarray(w_qkv, dtype=np.float32)
    w_proj = np.asarray(w_proj, dtype=np.float32)

    run = _get_runner()
    cosT, sinT = _rope_tables()

    wq = np.ascontiguousarray(w_qkv[:, :2 * DIM]).astype(np.float16)
    wv = np.ascontiguousarray(w_qkv[:, 2 * DIM:]).astype(np.float16)
    wp = w_proj.astype(np.float16)

    pmat = _perm_matrix()
    in_maps = []
    for b in range(NCORES):
        xTb = np.ascontiguousarray(x[b].T).astype(np.float16)      # (512, 1024)
        bias_b = np.where(mask[b].reshape(8, 128).T, 0.0, -1e9).astype(np.float32)
        in_maps.append({
            "xT": xTb, "wq": wq, "wv": wv, "wp": wp,
            "cosT": cosT, "sinT": sinT, "bias": bias_b, "pmat": pmat,
        })

    results = run(in_maps)
    out = np.stack([results[c]["y"] for c in range(NCORES)], axis=0)
    return out.astype(np.float32)


# revision 30
# speedup vs baseline: 1.2890x; 1.1510x over previous
"""Multi-head self-attention (RoPE) Trainium2 Bass kernel.

Shards batch (B=8) across 8 NeuronCores, one batch element per core.
Per core: fused qkv projection (fp16 matmuls), RoPE, flash-style attention
(scores row-tiled per head pair, exp on ACT with fused scale+mask-bias,
AV row-tiled by k-parity with a fused ones-column producing softmax
denominators), reciprocal-normalize, output projection.

KREPS body repetitions are emitted as a single hardware loop (tc.For_i)
so program size — and hence BIR->NEFF compile time — is independent of
the rep count; rep-count differencing then isolates true HW exec time.
"""
import os
import sys

# The kernel needs the 8 axon-tunneled NeuronCores visible to jax; a
# JAX_PLATFORMS=cpu pin (used by some harnesses for the reference) would
# hide them. Clear it before jax initializes through the concourse imports.
os.environ.pop("JAX_PLATFORMS", None)

sys.path.insert(0, "/opt/trn_rl_repo")

_REPS = int(os.environ.get("KREPS", "1"))
_DBG = os.environ.get("KDBG", "")

import numpy as np
from contextlib import ExitStack

import concourse.bass as bass
import concourse.tile as tile
from concourse import bacc, mybir
from concourse.tile import add_dep_helper

f32 = mybir.dt.float32
f16 = mybir.dt.float16
f8 = mybir.dt.float8e4
AF = mybir.ActivationFunctionType
ALU = mybir.AluOpType
DROW = mybir.MatmulPerfMode.DoubleRow

B, L, DIM = 8, 1024, 512
NH, HD = 8, 64
SCALE = HD ** -0.5
NCORES = 8


def _build_nc(reps=None):
    if reps is None:
        reps = _REPS
    nc = bacc.Bacc("TRN2", target_bir_lowering=False, debug=False, enable_asserts=False)

    xT = nc.dram_tensor("xT", (DIM, L), f16, kind="ExternalInput")
    wq = nc.dram_tensor("wq", (DIM, 2 * DIM), f16, kind="ExternalInput")  # Q|K cols
    wv = nc.dram_tensor("wv", (DIM, DIM), f16, kind="ExternalInput")      # V cols
    wp = nc.dram_tensor("wp", (DIM, DIM), f16, kind="ExternalInput")
    cosT = nc.dram_tensor("cosT", (128, L), f16, kind="ExternalInput")
    sinT = nc.dram_tensor("sinT", (128, L), f16, kind="ExternalInput")
    bias = nc.dram_tensor("bias", (128, 8), f32, kind="ExternalInput")
    pmat = nc.dram_tensor("pmat", (128, 128), f16, kind="ExternalInput")
    y = nc.dram_tensor("y", (L, DIM), f32, kind="ExternalOutput")

    with ExitStack() as ctx:
        tc = ctx.enter_context(tile.TileContext(nc))
        cst = ctx.enter_context(tc.tile_pool(name="cst", bufs=1))
        sc = ctx.enter_context(tc.tile_pool(name="sc", bufs=3))
        pTp = ctx.enter_context(tc.tile_pool(name="pTp", bufs=24))
        nrm = ctx.enter_context(tc.tile_pool(name="nrm", bufs=2))
        ysb = ctx.enter_context(tc.tile_pool(name="ysb", bufs=1))

        # ---- load inputs ----
        xT_all = cst.tile([128, 4 * L], f16, name="t", tag="xTall")
        wq_all = cst.tile([128, 4 * 2 * DIM], f16, name="t", tag="wqall")
        wv_all = cst.tile([128, 4 * DIM], f16, name="t", tag="wvall")
        wp_all = cst.tile([128, 4 * DIM], f16, name="t", tag="wpall")
        for big, dram, w in ((xT_all, xT, L), (wq_all, wq, 2 * DIM),
                             (wv_all, wv, DIM), (wp_all, wp, DIM)):
            nc.sync.dma_start(
                big[:].rearrange("p (kc w) -> p kc w", kc=4),
                dram[:].rearrange("(kc p) w -> p kc w", p=128))
        # 3D kc-major views for DoubleRow pair APs
        xT_3d = xT_all[:].rearrange("p (kc w) -> p kc w", kc=4)
        wq_3d = wq_all[:].rearrange("p (kc w) -> p kc w", kc=4)
        wv_3d = wv_all[:].rearrange("p (kc w) -> p kc w", kc=4)
        wp_sb = [wp_all[:, i * DIM:(i + 1) * DIM] for i in range(4)]
        cos_sb = cst.tile([128, L], f16, name="t", tag="cos")
        sin_sb = cst.tile([128, L], f16, name="t", tag="sin")
        bias_sb = cst.tile([128, 8], f32, name="t", tag="bias")
        pmat_sb = cst.tile([128, 128], f16, name="t", tag="pmat")
        nc.sync.dma_start(cos_sb[:], cosT[:])
        nc.sync.dma_start(sin_sb[:], sinT[:])
        nc.sync.dma_start(bias_sb[:], bias[:])
        nc.sync.dma_start(pmat_sb[:], pmat[:])

        # qkTfull[m]: QK-col block m after RoPE; partitions 0:64 = head 2m,
        # 64:128 = head 2m+1 (m 0-3 = Q heads 0-7, m 4-7 = K heads 0-7)
        qkTfull = [cst.tile([128, L], f16, name="t", tag=f"qkT{m}") for m in range(8)]
        vaug = [cst.tile([128, NH * 128], f16, name="t", tag=f"vaug{i}") for i in range(8)]
        outT = [cst.tile([128, L], f16, name="t", tag=f"outT{c}") for c in range(4)]

        # ones columns of vaug are rep-invariant: set them once, pre-loop
        for lb in range(8):
            ones_ap = vaug[lb][:].rearrange("p (h c) -> p h c", h=NH)[:, :, 64:128]
            nc.vector.memset(ones_ap, 1.0)

        def emit_body(rep):
            # ---------- phase 1: qkv projection + RoPE ----------
            # Per m-block: PE does 8 qkv matmuls + 2 rotate-half permutation
            # matmuls; ACT drains psum->f16; DVE does cos/sin muls + add.
            # P-matmul/DVE chain for block m is emitted under block m+1's
            # matmuls (software pipeline) so PE never waits on ACT.
            with tc.tile_pool(name=f"qkps{rep}", bufs=2, space="PSUM") as qk_ps, \
                 tc.tile_pool(name=f"swps{rep}", bufs=1, space="PSUM") as sw_ps, \
                 tc.tile_pool(name=f"vps{rep}", bufs=2, space="PSUM") as v_ps:

                def rope_tail(m, qc):
                    swp = sw_ps.tile([128, L], f32, name="t", tag="swps")
                    for qb in range(2):
                        nc.tensor.matmul(
                            swp[:, qb * 512:(qb + 1) * 512],
                            pmat_sb[:],
                            qc[:, qb * 512:(qb + 1) * 512],
                            start=True, stop=True)
                    q1 = sc.tile([128, L], f16, name="t", tag="q1")
                    nc.vector.tensor_mul(q1[:], qc[:], cos_sb[:])
                    q2 = sc.tile([128, L], f16, name="t", tag="q2")
                    if "q" in _DBG:
                        swsb = sc.tile([128, L], f16, name="t", tag="swsb")
                        nc.scalar.activation(swsb[:], swp[:], AF.Copy)
                        nc.vector.tensor_mul(q2[:], swsb[:], sin_sb[:])
                    else:
                        nc.vector.tensor_mul(q2[:], swp[:], sin_sb[:])
                    nc.vector.tensor_add(qkTfull[m][:], q1[:], q2[:])

                prev = None  # (m, qc)
                for m in (0, 4, 1, 5, 2, 6, 3, 7):
                    ps = qk_ps.tile([128, L], f32, name="t", tag="qkps")
                    for kc in range(4):
                        for qb in range(2):
                            nc.tensor.matmul(
                                ps[:, qb * 512:(qb + 1) * 512],
                                wq_3d[:, kc, m * 128:(m + 1) * 128],
                                xT_3d[:, kc, qb * 512:(qb + 1) * 512],
                                start=(kc == 0), stop=(kc == 3))
                    qc = sc.tile([128, L], f16, name="t", tag="qc")
                    nc.scalar.activation(qc[:], ps[:], AF.Copy)
                    if prev is not None:
                        rope_tail(*prev)
                    prev = (m, qc)

                for lb in range(8):
                    vps = v_ps.tile([128, DIM], f32, name="t", tag="vps")
                    for kc in range(4):
                        nc.tensor.matmul(
                            vps[:],
                            xT_3d[:, kc, lb * 128:(lb + 1) * 128],
                            wv_3d[:, kc, :],
                            start=(kc == 0), stop=(kc == 3))
                    if lb == 0:
                        rope_tail(*prev)
                        prev = None
                    out_ap = vaug[lb][:].rearrange("p (h c) -> p h c", h=NH)[:, :, 0:64]
                    in_ap = vps[:].rearrange("p (h c) -> p h c", h=NH)
                    if "v" in _DBG:
                        nc.vector.tensor_copy(out_ap, in_ap)
                    else:
                        nc.scalar.activation(out_ap, in_ap, AF.Copy)

            # ---------- phase 2: attention ----------
            # scores(p) tiles [128 k-pos, 1024 q] double-buffered; ACT does
            # exp with per-k-block mask bias; AV(p-1) matmul pairs are
            # interleaved between scores tiles to keep PE busy while ACT
            # drains; denominators come from vaug's fused ones columns.
            with tc.tile_pool(name=f"sps{rep}", bufs=2, space="PSUM") as s_ps, \
                 tc.tile_pool(name=f"avps{rep}", bufs=2, space="PSUM") as av_ps:

                def q_ap(h):
                    return qkTfull[h // 2][64 * (h % 2):64 * (h % 2) + 64, :]

                def k_ap(h):
                    return qkTfull[4 + h // 2][64 * (h % 2):64 * (h % 2) + 64, :]

                def av_steps(p, pts):
                    """Generator: 16 steps of 2 AV matmuls; normalize after
                    each head completes."""
                    for t in range(2):
                        h = 2 * p + t
                        X = av_ps.tile([128, L], f32, name="t", tag="avX")
                        for kc in range(8):
                            pt = pts[(t, kc)]
                            va = vaug[kc][:, h * 128:(h + 1) * 128]
                            for qb in range(2):
                                nc.tensor.matmul(
                                    X[:, qb * 512:(qb + 1) * 512], va,
                                    pt[:, qb * 512:(qb + 1) * 512],
                                    start=(kc == 0), stop=(kc == 7))
                            yield
                        D = nrm.tile([64, L], f32, name="t", tag="D")
                        nc.vector.tensor_copy(D[:], X[64:128, :])
                        R = nrm.tile([64, L], f32, name="t", tag="R")
                        nc.vector.reciprocal_approx_fast(R[:], D[:])
                        if t == 0:
                            nc.vector.tensor_mul(outT[p][0:64, :], X[0:64, :], R[:])
                        else:
                            tmp = nrm.tile([64, L], f16, name="t", tag="tmp")
                            nc.vector.tensor_mul(tmp[:], X[0:64, :], R[:])
                            nc.sync.dma_start(outT[p][64:128, :], tmp[:])

                prev = None  # generator of av steps for p-1
                for p in range(4):
                    pts = {}
                    for j in range(16):
                        t, kb = divmod(j, 8)
                        h = 2 * p + t
                        QT, KT = q_ap(h), k_ap(h)
                        s = s_ps.tile([128, 1024], f32, name="t", tag="s")
                        for qb in range(2):
                            nc.tensor.matmul(
                                s[:, qb * 512:(qb + 1) * 512],
                                KT[:, kb * 128:(kb + 1) * 128],
                                QT[:, qb * 512:(qb + 1) * 512],
                                start=True, stop=True)
                        pt = pTp.tile([128, 1024], f16, name="t", tag="pT")
                        nc.scalar.activation(pt[:], s[:], AF.Exp,
                                             bias=bias_sb[:, kb:kb + 1],
                                             scale=SCALE)
                        pts[(t, kb)] = pt
                        if prev is not None:
                            next(prev, None)
                    if prev is not None:
                        for _ in prev:   # finish av(p-1) tail (t=1 normalize)
                            pass
                    prev = av_steps(p, pts)
                for _ in prev:
                    pass

            # ---------- phase 3: output projection ----------
            with tc.tile_pool(name=f"yps{rep}", bufs=2, space="PSUM") as y_ps:
                yall = ysb.tile([128, 8 * DIM], f32, name="t", tag="yall")
                for lb in range(8):
                    yp = y_ps.tile([128, DIM], f32, name="t", tag="yps")
                    for c in range(4):
                        nc.tensor.matmul(
                            yp[:],
                            outT[c][:, lb * 128:(lb + 1) * 128],
                            wp_sb[c][:],
                            start=(c == 0), stop=(c == 3))
                    ysl = yall[:, lb * DIM:(lb + 1) * DIM]
                    nc.scalar.activation(ysl, yp[:], AF.Copy)
                    # per-block store overlaps the remaining proj matmuls
                    nc.sync.dma_start(y[lb * 128:(lb + 1) * 128, :], ysl)

        if reps == 1:
            emit_body(0)
        else:
            with tc.For_i(0, reps) as _i:
                emit_body(0)

    nc.compile()
    return nc


def _perm_matrix():
    """[128,128] f16: out[i] = in[(i%64+32)%64 + 64*(i//64)] — rotate_half's
    partition permutation (sign lives in sinT)."""
    P = np.zeros((128, 128), dtype=np.float16)
    for i in range(128):
        src = (i % 64 + 32) % 64 + 64 * (i // 64)
        P[src, i] = 1.0
    return P


def _rope_tables():
    inv_freq = 1.0 / (10000.0 ** (np.arange(0, HD, 2, dtype=np.float32) / HD))
    t = np.arange(L, dtype=np.float32)
    freqs = np.outer(t, inv_freq)                      # (L, 32)
    emb = np.concatenate([freqs, freqs], axis=-1)      # (L, 64)
    cos = np.cos(emb).T                                # (64, L)
    sin = np.sin(emb).T                                # (64, L)
    sign = np.where(np.arange(HD) < HD // 2, -1.0, 1.0)[:, None].astype(np.float32)
    sin_s = sin * sign
    cosT = np.tile(cos, (2, 1)).astype(np.float16)     # (128, L)
    sinT = np.tile(sin_s, (2, 1)).astype(np.float16)   # (128, L)
    return cosT, sinT


def _make_runner(nc, n_cores=NCORES):
    """Reusable jitted SPMD runner (replicates run_bass_kernel_spmd's axon
    redirect, but caches the jax.jit so repeat calls skip retrace +
    BIR->NEFF recompile + executable reload)."""
    import jax
    from jax.sharding import Mesh, PartitionSpec
    from jax.experimental.shard_map import shard_map
    from concourse import bass2jax

    bass2jax.install_neuronx_cc_hook()
    partition_name = nc.partition_id_tensor.name if nc.partition_id_tensor else None
    in_names, out_names, out_avals, zero_outs = [], [], [], []
    for alloc in nc.m.functions[0].allocations:
        if not isinstance(alloc, mybir.MemoryLocationSet):
            continue
        name = alloc.memorylocations[0].name
        if alloc.kind == "ExternalInput":
            if name != partition_name:
                in_names.append(name)
        elif alloc.kind == "ExternalOutput":
            shape = tuple(alloc.tensor_shape)
            dtype = mybir.dt.np(alloc.dtype)
            out_names.append(name)
            out_avals.append(jax.core.ShapedArray(shape, dtype))
            zero_outs.append(np.zeros(shape, dtype))
    n_params = len(in_names)
    n_outs = len(out_avals)
    all_in = list(in_names) + list(out_names)
    if partition_name is not None:
        all_in.append(partition_name)

    def _body(*args):
        operands = list(args)
        if partition_name is not None:
            operands.append(bass2jax.partition_id_tensor())
        outs = bass2jax._bass_exec_p.bind(
            *operands,
            out_avals=tuple(out_avals),
            in_names=tuple(all_in),
            out_names=tuple(out_names),
            lowering_input_output_aliases=(),
            sim_require_finite=True,
            sim_require_nnan=True,
            nc=nc,
        )
        return tuple(outs)

    devices = jax.devices()[:n_cores]
    mesh = Mesh(np.asarray(devices), ("core",))
    in_specs = (PartitionSpec("core"),) * (n_params + n_outs)
    out_specs = (PartitionSpec("core"),) * n_outs
    fn = jax.jit(
        shard_map(_body, mesh=mesh, in_specs=in_specs, out_specs=out_specs,
                  check_rep=False),
        keep_unused=True,
    )
    concat_zeros = [np.zeros((n_cores * z.shape[0], *z.shape[1:]), z.dtype)
                    for z in zero_outs]

    def run(in_maps):
        per_core = [[np.asarray(m[name]) for name in in_names] for m in in_maps]
        concat_in = [np.concatenate([per_core[c][i] for c in range(n_cores)], axis=0)
                     for i in range(n_params)]
        out_arrs = fn(*concat_in, *concat_zeros)
        return [
            {name: np.asarray(out_arrs[i]).reshape(n_cores, *out_avals[i].shape)[c]
             for i, name in enumerate(out_names)}
            for c in range(n_cores)
        ]

    run.fn = fn
    run.in_names = in_names
    run.concat_zeros = concat_zeros
    return run


def _make_runner_dev(nc, in_maps, n_cores=NCORES):
    """Timing variant of _make_runner: inputs pre-transferred to device so
    steady-state calls measure dispatch + device execution only."""
    import jax
    from jax.sharding import Mesh, PartitionSpec, NamedSharding

    run = _make_runner(nc, n_cores)
    fn = run.fn
    in_names = run.in_names
    concat_zeros = run.concat_zeros
    per_core = [[np.asarray(m[name]) for name in in_names] for m in in_maps]
    concat_in = [np.concatenate([per_core[c][i] for c in range(n_cores)], axis=0)
                 for i in range(len(in_names))]
    devices = jax.devices()[:n_cores]
    mesh = Mesh(np.asarray(devices), ("core",))
    sh = NamedSharding(mesh, PartitionSpec("core"))
    dev_args = [jax.device_put(a, sh) for a in concat_in + concat_zeros]

    def run_dev():
        outs = fn(*dev_args)
        jax.block_until_ready(outs)
        return outs

    return run_dev


_NC = None
_RUN = None


def _get_runner():
    global _NC, _RUN
    if _RUN is None:
        _NC = _build_nc()
        try:
            _RUN = _make_runner(_NC)
        except Exception:
            from concourse.bass_utils import run_bass_kernel_spmd

            def _RUN_fallback(in_maps):
                res = run_bass_kernel_spmd(_NC, in_maps,
                                           core_ids=list(range(NCORES)))
                return res.results
            _RUN = _RUN_fallback
    return _RUN


def kernel(x, mask, w_qkv, w_proj):
    x = np.asarray(x, dtype=np.float32)
    mask = np.asarray(mask)
    w_qkv = np.asarray(w_qkv, dtype=np.float32)
    w_proj = np.asarray(w_proj, dtype=np.float32)

    run = _get_runner()
    cosT, sinT = _rope_tables()

    wq = np.ascontiguousarray(w_qkv[:, :2 * DIM]).astype(np.float16)
    wv = np.ascontiguousarray(w_qkv[:, 2 * DIM:]).astype(np.float16)
    wp = w_proj.astype(np.float16)

    pmat = _perm_matrix()
    in_maps = []
    for b in range(NCORES):
        xTb = np.ascontiguousarray(x[b].T).astype(np.float16)      # (512, 1024)
        bias_b = np.where(mask[b].reshape(8, 128).T, 0.0, -1e9).astype(np.float32)
        in_maps.append({
            "xT": xTb, "wq": wq, "wv": wv, "wp": wp,
            "cosT": cosT, "sinT": sinT, "bias": bias_b, "pmat": pmat,
        })

    results = run(in_maps)
    out = np.stack([results[c]["y"] for c in range(NCORES)], axis=0)
    return out.astype(np.float32)
